# revision 24
# baseline (speedup 1.0000x reference)
"""Trainium2 Bass kernel for ConvolutionalAttention2D (linear attention with 1x1 convs).

Reference computation (per batch b):
    q = Wq x ; k = Wk x ; v = Wv x          (1x1 convs == channel matmuls)
    phi(t) = elu(t) + 1
    qv = phi(q) @ phi(v)^T                  ([C, C] context matrix, contract over pixels)
    out = Wo (qv @ phi_k) + bo

Kernel strategy (8 NeuronCores, data-parallel over batch B=16 -> 2 batches/core):
  - Algebraic refactor: Wo (qv @ phi_k) == (Wo qv) @ phi_k.
  - q/k/v projections in fp8e4m3 with DoubleRow (K=256 contraction in one
    pass, 2 MACs/cell/cycle). Quantization noise averages out in the
    all-positive qv/attended contractions (measured ~2.7e-3 final rel err).
  - qv / Wo-fold / attended matmuls in bf16 (fp8 W2 would overflow and
    output-projection noise does not average down).
  - phi(t) = elu(t)+1 computed as max(min(exp(t), 1), t+1): one ACT Exp pass
    + ONE fused custom-DVE pass (registered below).
  - Out-copies (PSUM->SBUF + bias) placed on ACT/DVE per pattern knob.
"""

from contextlib import ExitStack

import numpy as np

import concourse.bacc as bacc
import concourse.tile as tile
from concourse import mybir
from concourse import bass_utils

B, C, H, W = 16, 256, 64, 64
HW = H * W
NCORES = 8
NB = B // NCORES  # batches per core

FP = mybir.dt.float32
BF = mybir.dt.bfloat16
F8 = mybir.dt.float8e4
AF = mybir.ActivationFunctionType
OP = mybir.AluOpType
DR = mybir.MatmulPerfMode.DoubleRow


def _register_phi_op():
    """Register a fused DVE op: out = max(min(in0, s0), in1 + s1).

    With in0 = exp(x) and s0 = s1 = 1 this is exactly phi(x) = elu(x)+1:
    for x <= 0, e^x >= 1+x so max picks e^x (and e^x <= 1); for x > 0,
    min clamps to 1 and max picks 1+x.
    """
    from concourse import dve_ops as D
    from concourse.dve_spec import Spec, Src0, Src1, C0, C1, maxx, minn, lower, _has_src1
    from concourse.dve_uop import DveOpSpec

    name = "PHI_COMBINE_ANT"
    for op in D.OPS:
        if op.name == name:
            return op
    spec = Spec(
        body=maxx(minn(Src0, C0), Src1 + C1),
        reference=lambda in0, in1, s0, s1, imm2: np.maximum(
            np.minimum(in0.astype(np.float32), s0), in1.astype(np.float32) + s1
        ),
    )
    shas = {}
    for ver in ("v3", "v4"):
        u = lower(spec, ver=ver)
        shas[ver] = DveOpSpec(
            name=name, opcode=0, uops=u, rd1_en=_has_src1(spec)
        ).sha(ver)
    op = D.DveOp(name, spec, subdim=False, uops_sha=shas)
    D.OPS.append(op)
    D._SUB_OPCODE_FOR_NAME[name] = D._CUSTOM_DVE_ROW_BASE + len(D.OPS) - 1
    D.CUSTOM_DVE_SPECS[name] = spec
    return op


PHI_OP = _register_phi_op()

# per-batch span schemes: 24 spans (16 stage-B + 8 stage-A)
#  'f': ACT Exp ; fused DVE max(min(e,1), psum+1)             [default]
#  'b': ACT Exp ; ACT Relu ; DVE stt(e min 1 + r) all-bf16
DEFAULT_SPANS = "f" * 24
# per-batch out-copy engine for the 8 stage-D tiles: 'A' = ACT w/ bias, 'V' = DVE
DEFAULT_OUTS = "AAAAAVAVAVAVAVAV"
# software-pipelined stage emission order (PE executes in program order, so
# batch-1 projections are emitted before batch-0 consumers to keep ACT/DVE fed)
PIPELINE = ["x0", "B0", "A0", "C0", "D0", "x1", "B1", "A1", "C1", "D1"]
OUT_GROUP = 2  # stage-D out-copies per DMA (1, 2, or 4)
OUT_QUEUES = "SSSSSSAA"  # issue queue per out-DMA (S=SP, P=Pool, A=ACT)
X_QUEUE = "S"  # issue queue for x loads
UNROLL = 8  # bodies per For_i trip
QV_COPY_ENG = "A"  # qv/w2 PSUM->SBUF copies: ACT or DVE (barrier amortization + cross-body overlap)


def build_kernel(repeat: int = 1, span_pattern=DEFAULT_SPANS, out_pattern=DEFAULT_OUTS,
                 xp_bufs=2, phikp_bufs=2, pqvp_bufs=2, mm_bufs=3, tmps_bufs=6,
                 outp_bufs=4, out_fp32=False, pipeline=None):
    """Build the per-core Bass program. `repeat` wraps the whole body in a
    dynamic For_i loop (used only for wall-clock timing runs)."""
    nc = bacc.Bacc("TRN2", target_bir_lowering=False, debug=False)

    # x packed [NB, cc, 128, HW] fp8; projection weights [cc, 128, wq|wv|wk]
    x_d = nc.dram_tensor("x", [NB, 2, 128, HW], F8, kind="ExternalInput")
    w8_d = nc.dram_tensor("w8", [2, 128, 768], F8, kind="ExternalInput")
    wo_d = nc.dram_tensor("wo", [2, 128, 256], BF, kind="ExternalInput")
    bo_d = nc.dram_tensor("bo", [C, 1], FP, kind="ExternalInput")
    OD = FP if out_fp32 else BF
    out_d = nc.dram_tensor("out", [NB, C, HW], OD, kind="ExternalOutput")

    with tile.TileContext(nc) as tc, ExitStack() as ctx:
        singles = ctx.enter_context(tc.tile_pool(name="singles", bufs=1))
        xp = ctx.enter_context(tc.tile_pool(name="xp", bufs=xp_bufs))
        phikp = ctx.enter_context(tc.tile_pool(name="phikp", bufs=phikp_bufs))
        pqvp = ctx.enter_context(tc.tile_pool(name="pqvp", bufs=pqvp_bufs))
        tmps = ctx.enter_context(tc.tile_pool(name="tmps", bufs=tmps_bufs))
        smalls = ctx.enter_context(tc.tile_pool(name="smalls", bufs=2))
        outp = ctx.enter_context(tc.tile_pool(name="outp", bufs=outp_bufs))
        psmm = ctx.enter_context(tc.tile_pool(name="psmm", bufs=mm_bufs, space="PSUM"))
        psacc = ctx.enter_context(tc.tile_pool(name="psacc", bufs=1, space="PSUM"))

        # ---- weights (loaded once, replicated) ----
        w8_all = singles.tile([128, 2, 768], F8, tag="w8")
        nc.sync.dma_start(out=w8_all[:], in_=w8_d.ap().rearrange("cc p b -> p cc b"))
        wqv8 = w8_all[:, :, 0:512]     # [128, 2, 512] DoubleRow rhs (wq|wv)
        wk8 = w8_all[:, :, 512:768]    # [128, 2, 256] DoubleRow lhsT source
        wo_sb = singles.tile([128, 2, 256], BF, tag="wo")
        nc.sync.dma_start(out=wo_sb[:], in_=wo_d.ap().rearrange("cc p b -> p cc b"))
        wo = [wo_sb[:, cc, :] for cc in range(2)]
        bo_sb = singles.tile([128, 2], FP, tag="bo")
        for m in range(2):
            nc.sync.dma_start(
                out=bo_sb[:, m:m + 1], in_=bo_d.ap()[m * 128:(m + 1) * 128, :]
            )

        state = {"span": 0, "out": 0}

        def phi_span(psum3_ap, dst_ap, dst_bank2=None):
            """dst = phi(psum) = max(min(exp(x), 1), x + 1), bf16 out.

            psum3_ap must be rank-3 [128, a, b] (the custom-DVE STT struct
            requires a 2-free-dim src1); dst free size must match."""
            i = state["span"] % 24
            state["span"] += 1
            sch = span_pattern[i]
            flat = psum3_ap.rearrange("p a b -> p (a b)")
            e = tmps.tile([128, 1024], BF, tag="e")
            nc.scalar.activation(e[:], flat, AF.Exp)
            if sch == "b":
                r = tmps.tile([128, 1024], BF, tag="r")
                nc.scalar.activation(r[:], flat, AF.Relu)
                nc.vector.scalar_tensor_tensor(dst_ap, e[:], 1.0, r[:], OP.min, OP.add)
            else:
                nc.vector._custom_dve(
                    PHI_OP, out=dst_ap, in0=e[:], in1=psum3_ap, s0=1.0, s1=1.0
                )

        def body(_iv=None):
            state["span"] = 0
            state["out"] = 0
            st = [dict() for _ in range(NB)]
            XQ = {"S": nc.sync, "P": nc.gpsimd, "A": nc.scalar}[X_QUEUE]

            def stage_x(b):
                X8 = xp.tile([128, 2, HW], F8, tag="x", name=f"x{b}")
                st[b]["X8"] = X8
                xblocks = [(0, 512), (512, 1536), (2048, 2048)]
                for (c0, cw) in xblocks:
                    cs = slice(c0, c0 + cw)
                    XQ.dma_start(
                        out=X8[:, :, cs],
                        in_=x_d.ap()[b, :, :, cs].rearrange("cc p n -> p cc n"),
                    )

            def stage_B(b):
                X8 = st[b]["X8"]

                # ---- stage B: phi(q^T), phi(v^T), transposed layout [n, o] ----
                # DoubleRow: stationary = x chunk [128,2,128], moving = wq|wv
                pqv_sb = pqvp.tile([128, 32, 512], BF, tag="pqv")
                st[b]["pqv"] = pqv_sb
                for i in range(16):
                    ps = psmm.tile([128, 2, 512], FP, tag="mm")
                    for j in range(2):
                        nk = i * 2 + j
                        nc.tensor.matmul(
                            ps[:, j, :],
                            X8[:, :, nk * 128:(nk + 1) * 128],
                            wqv8,
                            start=True,
                            stop=True,
                            perf_mode=DR,
                        )
                    phi_span(ps[:], pqv_sb[:, i * 2:(i + 1) * 2, :])

            def stage_A(b):
                X8 = st[b]["X8"]
                # ---- stage A: phi_k [128, dd, 4096] (d = dd*128 + row) ----
                phik = phikp.tile([128, 2, HW], BF, tag="phik")
                st[b]["phik"] = phik
                for m in range(2):
                    for i in range(4):
                        ps = psmm.tile([128, 2, 512], FP, tag="mm")
                        for j in range(2):
                            n0 = (i * 2 + j) * 512
                            nc.tensor.matmul(
                                ps[:, j, :],
                                wk8[:, :, m * 128:(m + 1) * 128],
                                X8[:, :, n0:n0 + 512],
                                start=True,
                                stop=True,
                                perf_mode=DR,
                            )
                        phi_span(ps[:], phik[:, m, i * 1024:(i + 1) * 1024])

            def stage_C(b):
                pqv_sb = st[b]["pqv"]
                # ---- stage C: qv[c, d] = sum_n phi_qT[n, c] phi_vT[n, d] ----
                # interleaved cc accumulation chains -> each in its own bank
                qv_ps = psacc.tile([128, 2, 512], FP, tag="acc")
                for i in range(32):
                    for cc in range(2):
                        nc.tensor.matmul(
                            qv_ps[:, cc, 0:256],
                            pqv_sb[:, i, cc * 128:(cc + 1) * 128],
                            pqv_sb[:, i, 256:512],
                            start=(i == 0),
                            stop=(i == 31),
                        )
                qv_sb = smalls.tile([128, 2, 256], BF, tag="qv_sb")
                if QV_COPY_ENG == "A":
                    nc.scalar.activation(qv_sb[:], qv_ps[:, :, 0:256], AF.Copy)
                else:
                    nc.vector.tensor_copy(qv_sb[:], qv_ps[:, :, 0:256])

                # ---- stage C2: W2^T[d, o] = sum_c qv[c, d] WoT[c, o] ----
                w2_ps = psacc.tile([128, 2, 256], FP, tag="acc")
                for dd in range(2):
                    for cc in range(2):
                        nc.tensor.matmul(
                            w2_ps[:, dd, :],
                            qv_sb[:, cc, dd * 128:(dd + 1) * 128],
                            wo[cc],
                            start=(cc == 0),
                            stop=(cc == 1),
                        )
                w2_sb = smalls.tile([128, 2, 256], BF, tag="w2_sb")
                st[b]["w2"] = w2_sb
                if QV_COPY_ENG == "A":
                    nc.scalar.activation(
                        w2_sb[:].rearrange("p a b -> p (a b)"),
                        w2_ps[:].rearrange("p a b -> p (a b)"),
                        AF.Copy,
                    )
                else:
                    nc.vector.tensor_copy(
                        w2_sb[:].rearrange("p a b -> p (a b)"),
                        w2_ps[:].rearrange("p a b -> p (a b)"),
                    )

            def stage_D(b):
                w2_sb = st[b]["w2"]
                phik = st[b]["phik"]
                # ---- stage D: out[o, n] = sum_d W2[o, d] phi_k[d, n] + bo ----
                G = OUT_GROUP
                for m in range(2):
                    for ip in range(4 // G):
                        o_sb = outp.tile([128, G, 1024], OD, tag="osb")
                        for ih in range(G):
                            i = ip * G + ih
                            ps = psmm.tile([128, 1024], FP, tag="mm")
                            for j in range(2):
                                n0 = (i * 2 + j) * 512
                                for dd in range(2):
                                    nc.tensor.matmul(
                                        ps[:, j * 512:(j + 1) * 512],
                                        w2_sb[:, dd, m * 128:(m + 1) * 128],
                                        phik[:, dd, n0:n0 + 512],
                                        start=(dd == 0),
                                        stop=(dd == 1),
                                    )
                            oe = out_pattern[state["out"] % len(out_pattern)]
                            if oe == "A":
                                nc.scalar.activation(
                                    o_sb[:, ih, :], ps[:], AF.Identity,
                                    bias=bo_sb[:, m:m + 1],
                                )
                            else:
                                nc.vector.tensor_scalar_add(
                                    o_sb[:, ih, :], ps[:], bo_sb[:, m:m + 1]
                                )
                            state["out"] += 1
                        qi = (state["out"] // G - 1) % len(OUT_QUEUES)
                        dma_eng = {"S": nc.sync, "P": nc.gpsimd,
                                   "A": nc.scalar, "V": nc.vector}[OUT_QUEUES[qi]]
                        dma_eng.dma_start(
                            out=out_d.ap()[b, m * 128:(m + 1) * 128,
                                           ip * G * 1024:(ip + 1) * G * 1024],
                            in_=o_sb[:].rearrange("p a b -> p (a b)"),
                        )

            stages = {"x": stage_x, "B": stage_B, "A": stage_A,
                      "C": stage_C, "D": stage_D}
            for tok in (pipeline or PIPELINE):
                stages[tok[0]](int(tok[1]))

        if repeat == 1:
            body()
        else:
            assert repeat % UNROLL == 0, (repeat, UNROLL)
            with tc.For_i(0, repeat // UNROLL, 1) as iv:
                for _u in range(UNROLL):
                    body(iv)

    nc.compile()
    return nc


_nc_cache = {}


def _get_nc(repeat: int = 1):
    if repeat not in _nc_cache:
        _nc_cache[repeat] = build_kernel(repeat)
    return _nc_cache[repeat]


def _cast(a, dt):
    import ml_dtypes  # noqa: F401
    return np.asarray(a, dtype=np.float32).astype(dt)


F8NP = mybir.dt.np(F8)
BFNP = mybir.dt.np(BF)


def make_in_maps(x, Wq, Wk, Wv, Wo, bo):
    x = np.asarray(x, dtype=np.float32).reshape(B, 2, 128, HW)
    x8 = np.ascontiguousarray(_cast(x, F8NP))
    wq_t = np.asarray(Wq, dtype=np.float32).T.reshape(2, 128, C)
    wv_t = np.asarray(Wv, dtype=np.float32).T.reshape(2, 128, C)
    wk_t = np.asarray(Wk, dtype=np.float32).T.reshape(2, 128, C)
    wo_t = np.asarray(Wo, dtype=np.float32).T.reshape(2, 128, C)
    w8 = np.ascontiguousarray(_cast(np.concatenate([wq_t, wv_t, wk_t], axis=2), F8NP))
    wob = np.ascontiguousarray(_cast(wo_t, BFNP))
    bo2 = np.ascontiguousarray(np.asarray(bo, dtype=np.float32).reshape(C, 1))
    return [
        {"x": x8[i * NB:(i + 1) * NB], "w8": w8, "wo": wob, "bo": bo2}
        for i in range(NCORES)
    ]


def kernel(x, Wq, Wk, Wv, Wo, bo):
    nc = _get_nc(repeat=1)
    in_maps = make_in_maps(x, Wq, Wk, Wv, Wo, bo)
    res = bass_utils.run_bass_kernel_spmd(nc, in_maps, core_ids=list(range(NCORES)))
    out = np.concatenate([res.results[i]["out"] for i in range(NCORES)], axis=0)
    return np.ascontiguousarray(out.reshape(B, C, H, W).astype(np.float32))


# revision 25
# speedup vs baseline: 1.6022x; 1.6022x over previous
"""Trainium2 Bass kernel for ConvolutionalAttention2D (linear attention with 1x1 convs).

Reference computation (per batch b):
    q = Wq x ; k = Wk x ; v = Wv x          (1x1 convs == channel matmuls)
    phi(t) = elu(t) + 1
    qv = phi(q) @ phi(v)^T                  ([C, C] context matrix, contract over pixels)
    out = Wo (qv @ phi_k) + bo

Kernel strategy (8 NeuronCores, data-parallel over batch B=16 -> 2 batches/core):
  - Algebraic refactor: Wo (qv @ phi_k) == (Wo qv) @ phi_k.
  - q/k/v projections in fp8e4m3 with DoubleRow (K=256 contraction in one
    pass, 2 MACs/cell/cycle). Quantization noise averages out in the
    all-positive qv/attended contractions (measured ~2.7e-3 final rel err).
  - qv / Wo-fold / attended matmuls in bf16 (fp8 W2 would overflow and
    output-projection noise does not average down).
  - phi(t) = elu(t)+1 computed as max(min(exp(t), 1), t+1): one ACT Exp pass
    + ONE fused custom-DVE pass (registered below).
  - Out-copies (PSUM->SBUF + bias) placed on ACT/DVE per pattern knob.
"""

from contextlib import ExitStack

import numpy as np

import concourse.bacc as bacc
import concourse.tile as tile
from concourse import mybir
from concourse import bass_utils

B, C, H, W = 16, 256, 64, 64
HW = H * W
NCORES = 8
NB = B // NCORES  # batches per core

FP = mybir.dt.float32
BF = mybir.dt.bfloat16
F8 = mybir.dt.float8e4
AF = mybir.ActivationFunctionType
OP = mybir.AluOpType
DR = mybir.MatmulPerfMode.DoubleRow


def _register_phi_op():
    """Register a fused DVE op: out = max(min(in0, s0), in1 + s1).

    With in0 = exp(x) and s0 = s1 = 1 this is exactly phi(x) = elu(x)+1:
    for x <= 0, e^x >= 1+x so max picks e^x (and e^x <= 1); for x > 0,
    min clamps to 1 and max picks 1+x.
    """
    from concourse import dve_ops as D
    from concourse.dve_spec import Spec, Src0, Src1, C0, C1, maxx, minn, lower, _has_src1
    from concourse.dve_uop import DveOpSpec

    name = "PHI_COMBINE_ANT"
    for op in D.OPS:
        if op.name == name:
            return op
    spec = Spec(
        body=maxx(minn(Src0, C0), Src1 + C1),
        reference=lambda in0, in1, s0, s1, imm2: np.maximum(
            np.minimum(in0.astype(np.float32), s0), in1.astype(np.float32) + s1
        ),
    )
    shas = {}
    for ver in ("v3", "v4"):
        u = lower(spec, ver=ver)
        shas[ver] = DveOpSpec(
            name=name, opcode=0, uops=u, rd1_en=_has_src1(spec)
        ).sha(ver)
    op = D.DveOp(name, spec, subdim=False, uops_sha=shas)
    D.OPS.append(op)
    D._SUB_OPCODE_FOR_NAME[name] = D._CUSTOM_DVE_ROW_BASE + len(D.OPS) - 1
    D.CUSTOM_DVE_SPECS[name] = spec
    return op


PHI_OP = _register_phi_op()

# per-batch span schemes: 24 spans (16 stage-B + 8 stage-A)
#  'f': ACT Exp ; fused DVE max(min(e,1), psum+1)             [default]
#  'b': ACT Exp ; ACT Relu ; DVE stt(e min 1 + r) all-bf16
DEFAULT_SPANS = "f" * 24
# per-batch out-copy engine for the 8 stage-D tiles: 'A' = ACT w/ bias, 'V' = DVE
DEFAULT_OUTS = "AAAAAVAVAVAVAVAV"
# software-pipelined stage emission order (PE executes in program order, so
# batch-1 projections are emitted before batch-0 consumers to keep ACT/DVE fed)
PIPELINE = ["x0", "B0", "A0", "C0", "D0", "x1", "B1", "A1", "C1", "D1"]
OUT_GROUP = 2  # stage-D out-copies per DMA (1, 2, or 4)
OUT_QUEUES = "SSSSSSAA"  # issue queue per out-DMA (S=SP, P=Pool, A=ACT)
X_QUEUE = "S"  # issue queue for x loads
UNROLL = 4  # bodies per For_i trip
QV_COPY_ENG = "A"  # qv/w2 PSUM->SBUF copies: ACT or DVE (barrier amortization + cross-body overlap)


def build_kernel(repeat: int = 1, span_pattern=DEFAULT_SPANS, out_pattern=DEFAULT_OUTS,
                 xp_bufs=2, phikp_bufs=2, pqvp_bufs=2, mm_bufs=3, tmps_bufs=6,
                 outp_bufs=4, out_fp32=False, pipeline=None):
    """Build the per-core Bass program. `repeat` wraps the whole body in a
    dynamic For_i loop (used only for wall-clock timing runs)."""
    nc = bacc.Bacc("TRN2", target_bir_lowering=False, debug=False)

    # x packed [NB, cc, 128, HW] fp8; projection weights [cc, 128, wq|wv|wk]
    x_d = nc.dram_tensor("x", [NB, 2, 128, HW], F8, kind="ExternalInput")
    w8_d = nc.dram_tensor("w8", [2, 128, 768], F8, kind="ExternalInput")
    wo_d = nc.dram_tensor("wo", [2, 128, 256], BF, kind="ExternalInput")
    bo_d = nc.dram_tensor("bo", [C, 1], FP, kind="ExternalInput")
    OD = FP if out_fp32 else BF
    out_d = nc.dram_tensor("out", [NB, C, HW], OD, kind="ExternalOutput")

    with tile.TileContext(nc) as tc, ExitStack() as ctx:
        singles = ctx.enter_context(tc.tile_pool(name="singles", bufs=1))
        xp = ctx.enter_context(tc.tile_pool(name="xp", bufs=xp_bufs))
        phikp = ctx.enter_context(tc.tile_pool(name="phikp", bufs=phikp_bufs))
        pqvp = ctx.enter_context(tc.tile_pool(name="pqvp", bufs=pqvp_bufs))
        tmps = ctx.enter_context(tc.tile_pool(name="tmps", bufs=tmps_bufs))
        smalls = ctx.enter_context(tc.tile_pool(name="smalls", bufs=2))
        outp = ctx.enter_context(tc.tile_pool(name="outp", bufs=outp_bufs))
        psmm = ctx.enter_context(tc.tile_pool(name="psmm", bufs=mm_bufs, space="PSUM"))
        psacc = ctx.enter_context(tc.tile_pool(name="psacc", bufs=1, space="PSUM"))

        # ---- weights (loaded once, replicated) ----
        w8_all = singles.tile([128, 2, 768], F8, tag="w8")
        nc.sync.dma_start(out=w8_all[:], in_=w8_d.ap().rearrange("cc p b -> p cc b"))
        wqv8 = w8_all[:, :, 0:512]     # [128, 2, 512] DoubleRow rhs (wq|wv)
        wk8 = w8_all[:, :, 512:768]    # [128, 2, 256] DoubleRow lhsT source
        wo_sb = singles.tile([128, 2, 256], BF, tag="wo")
        nc.sync.dma_start(out=wo_sb[:], in_=wo_d.ap().rearrange("cc p b -> p cc b"))
        wo = [wo_sb[:, cc, :] for cc in range(2)]
        bo_sb = singles.tile([128, 2], FP, tag="bo")
        for m in range(2):
            nc.sync.dma_start(
                out=bo_sb[:, m:m + 1], in_=bo_d.ap()[m * 128:(m + 1) * 128, :]
            )

        state = {"span": 0, "out": 0}

        def phi_span(psum3_ap, dst_ap, dst_bank2=None):
            """dst = phi(psum) = max(min(exp(x), 1), x + 1), bf16 out.

            psum3_ap must be rank-3 [128, a, b] (the custom-DVE STT struct
            requires a 2-free-dim src1); dst free size must match."""
            i = state["span"] % 24
            state["span"] += 1
            sch = span_pattern[i]
            flat = psum3_ap.rearrange("p a b -> p (a b)")
            e = tmps.tile([128, 1024], BF, tag="e")
            nc.scalar.activation(e[:], flat, AF.Exp)
            if sch == "b":
                r = tmps.tile([128, 1024], BF, tag="r")
                nc.scalar.activation(r[:], flat, AF.Relu)
                nc.vector.scalar_tensor_tensor(dst_ap, e[:], 1.0, r[:], OP.min, OP.add)
            else:
                nc.vector._custom_dve(
                    PHI_OP, out=dst_ap, in0=e[:], in1=psum3_ap, s0=1.0, s1=1.0
                )

        def body(_iv=None):
            state["span"] = 0
            state["out"] = 0
            st = [dict() for _ in range(NB)]
            XQ = {"S": nc.sync, "P": nc.gpsimd, "A": nc.scalar}[X_QUEUE]

            def stage_x(b):
                X8 = xp.tile([128, 2, HW], F8, tag="x", name=f"x{b}")
                st[b]["X8"] = X8
                xblocks = [(0, 512), (512, 1536), (2048, 2048)]
                for (c0, cw) in xblocks:
                    cs = slice(c0, c0 + cw)
                    XQ.dma_start(
                        out=X8[:, :, cs],
                        in_=x_d.ap()[b, :, :, cs].rearrange("cc p n -> p cc n"),
                    )

            def stage_B(b):
                X8 = st[b]["X8"]

                # ---- stage B: phi(q^T), phi(v^T), transposed layout [n, o] ----
                # DoubleRow: stationary = x chunk [128,2,128], moving = wq|wv
                pqv_sb = pqvp.tile([128, 32, 512], BF, tag="pqv")
                st[b]["pqv"] = pqv_sb
                for i in range(16):
                    ps = psmm.tile([128, 2, 512], FP, tag="mm")
                    for j in range(2):
                        nk = i * 2 + j
                        nc.tensor.matmul(
                            ps[:, j, :],
                            X8[:, :, nk * 128:(nk + 1) * 128],
                            wqv8,
                            start=True,
                            stop=True,
                            perf_mode=DR,
                        )
                    phi_span(ps[:], pqv_sb[:, i * 2:(i + 1) * 2, :])

            def stage_A(b):
                X8 = st[b]["X8"]
                # ---- stage A: phi_k [128, dd, 4096] (d = dd*128 + row) ----
                phik = phikp.tile([128, 2, HW], BF, tag="phik")
                st[b]["phik"] = phik
                for m in range(2):
                    for i in range(4):
                        ps = psmm.tile([128, 2, 512], FP, tag="mm")
                        for j in range(2):
                            n0 = (i * 2 + j) * 512
                            nc.tensor.matmul(
                                ps[:, j, :],
                                wk8[:, :, m * 128:(m + 1) * 128],
                                X8[:, :, n0:n0 + 512],
                                start=True,
                                stop=True,
                                perf_mode=DR,
                            )
                        phi_span(ps[:], phik[:, m, i * 1024:(i + 1) * 1024])

            def stage_C(b):
                pqv_sb = st[b]["pqv"]
                # ---- stage C: qv[c, d] = sum_n phi_qT[n, c] phi_vT[n, d] ----
                # interleaved cc accumulation chains -> each in its own bank
                qv_ps = psacc.tile([128, 2, 512], FP, tag="acc")
                for i in range(32):
                    for cc in range(2):
                        nc.tensor.matmul(
                            qv_ps[:, cc, 0:256],
                            pqv_sb[:, i, cc * 128:(cc + 1) * 128],
                            pqv_sb[:, i, 256:512],
                            start=(i == 0),
                            stop=(i == 31),
                        )
                qv_sb = smalls.tile([128, 2, 256], BF, tag="qv_sb")
                if QV_COPY_ENG == "A":
                    nc.scalar.activation(qv_sb[:], qv_ps[:, :, 0:256], AF.Copy)
                else:
                    nc.vector.tensor_copy(qv_sb[:], qv_ps[:, :, 0:256])

                # ---- stage C2: W2^T[d, o] = sum_c qv[c, d] WoT[c, o] ----
                w2_ps = psacc.tile([128, 2, 256], FP, tag="acc")
                for dd in range(2):
                    for cc in range(2):
                        nc.tensor.matmul(
                            w2_ps[:, dd, :],
                            qv_sb[:, cc, dd * 128:(dd + 1) * 128],
                            wo[cc],
                            start=(cc == 0),
                            stop=(cc == 1),
                        )
                w2_sb = smalls.tile([128, 2, 256], BF, tag="w2_sb")
                st[b]["w2"] = w2_sb
                if QV_COPY_ENG == "A":
                    nc.scalar.activation(
                        w2_sb[:].rearrange("p a b -> p (a b)"),
                        w2_ps[:].rearrange("p a b -> p (a b)"),
                        AF.Copy,
                    )
                else:
                    nc.vector.tensor_copy(
                        w2_sb[:].rearrange("p a b -> p (a b)"),
                        w2_ps[:].rearrange("p a b -> p (a b)"),
                    )

            def stage_D(b):
                w2_sb = st[b]["w2"]
                phik = st[b]["phik"]
                # ---- stage D: out[o, n] = sum_d W2[o, d] phi_k[d, n] + bo ----
                G = OUT_GROUP
                for m in range(2):
                    for ip in range(4 // G):
                        o_sb = outp.tile([128, G, 1024], OD, tag="osb")
                        for ih in range(G):
                            i = ip * G + ih
                            ps = psmm.tile([128, 1024], FP, tag="mm")
                            for j in range(2):
                                n0 = (i * 2 + j) * 512
                                for dd in range(2):
                                    nc.tensor.matmul(
                                        ps[:, j * 512:(j + 1) * 512],
                                        w2_sb[:, dd, m * 128:(m + 1) * 128],
                                        phik[:, dd, n0:n0 + 512],
                                        start=(dd == 0),
                                        stop=(dd == 1),
                                    )
                            oe = out_pattern[state["out"] % len(out_pattern)]
                            if oe == "A":
                                nc.scalar.activation(
                                    o_sb[:, ih, :], ps[:], AF.Identity,
                                    bias=bo_sb[:, m:m + 1],
                                )
                            else:
                                nc.vector.tensor_scalar_add(
                                    o_sb[:, ih, :], ps[:], bo_sb[:, m:m + 1]
                                )
                            state["out"] += 1
                        qi = (state["out"] // G - 1) % len(OUT_QUEUES)
                        dma_eng = {"S": nc.sync, "P": nc.gpsimd,
                                   "A": nc.scalar, "V": nc.vector}[OUT_QUEUES[qi]]
                        dma_eng.dma_start(
                            out=out_d.ap()[b, m * 128:(m + 1) * 128,
                                           ip * G * 1024:(ip + 1) * G * 1024],
                            in_=o_sb[:].rearrange("p a b -> p (a b)"),
                        )

            stages = {"x": stage_x, "B": stage_B, "A": stage_A,
                      "C": stage_C, "D": stage_D}
            for tok in (pipeline or PIPELINE):
                stages[tok[0]](int(tok[1]))

        if repeat == 1:
            body()
        else:
            assert repeat % UNROLL == 0, (repeat, UNROLL)
            with tc.For_i(0, repeat // UNROLL, 1) as iv:
                for _u in range(UNROLL):
                    body(iv)

    nc.compile()
    return nc


_nc_cache = {}


def _get_nc(repeat: int = 1):
    if repeat not in _nc_cache:
        _nc_cache[repeat] = build_kernel(repeat)
    return _nc_cache[repeat]


def _cast(a, dt):
    import ml_dtypes  # noqa: F401
    return np.asarray(a, dtype=np.float32).astype(dt)


F8NP = mybir.dt.np(F8)
BFNP = mybir.dt.np(BF)


def make_in_maps(x, Wq, Wk, Wv, Wo, bo):
    x = np.asarray(x, dtype=np.float32).reshape(B, 2, 128, HW)
    x8 = np.ascontiguousarray(_cast(x, F8NP))
    wq_t = np.asarray(Wq, dtype=np.float32).T.reshape(2, 128, C)
    wv_t = np.asarray(Wv, dtype=np.float32).T.reshape(2, 128, C)
    wk_t = np.asarray(Wk, dtype=np.float32).T.reshape(2, 128, C)
    wo_t = np.asarray(Wo, dtype=np.float32).T.reshape(2, 128, C)
    w8 = np.ascontiguousarray(_cast(np.concatenate([wq_t, wv_t, wk_t], axis=2), F8NP))
    wob = np.ascontiguousarray(_cast(wo_t, BFNP))
    bo2 = np.ascontiguousarray(np.asarray(bo, dtype=np.float32).reshape(C, 1))
    return [
        {"x": x8[i * NB:(i + 1) * NB], "w8": w8, "wo": wob, "bo": bo2}
        for i in range(NCORES)
    ]


def kernel(x, Wq, Wk, Wv, Wo, bo):
    nc = _get_nc(repeat=1)
    in_maps = make_in_maps(x, Wq, Wk, Wv, Wo, bo)
    res = bass_utils.run_bass_kernel_spmd(nc, in_maps, core_ids=list(range(NCORES)))
    out = np.concatenate([res.results[i]["out"] for i in range(NCORES)], axis=0)
    return np.ascontiguousarray(out.reshape(B, C, H, W).astype(np.float32))
